# revision 1
# baseline (speedup 1.0000x reference)
"""Trainium2 Bass kernel — latency-minimized DA-RNN input-attention encoder.

Cost model for this environment (axon-tunneled cores, measured by probe):
every STATIC program instruction costs ~30-200us per call (shipping/
translation), dynamic re-execution inside a hardware loop is nearly free,
cross-engine dependency handoffs cost ~20us each, and per-element device
time is secondary. The kernel therefore (a) runs the recurrence as a
tc.For_i hardware loop over 16-step windows (program ~1k static
instructions instead of ~28k), and (b) minimizes ops on the serial
dependency chain of each step (~22 instructions vs ~108 in the original).

Per-core dataflow (b = 64 batch rows, batch-major attention):
  preamble: ux[b, n, s] = sum_t X[b,t,n] Ue[t,s] + (be+bu)[s]  (256 matmuls)
  per step (For_i body unrolls 16 of these):
    hs[b,s] = [H;C]-contraction @ We_eff          2 PE  -> PSUM [64, 256]
    arg     = ux + hs[:,None,:]                   1 DVE (bf16, in-place buf)
    th      = tanh(arg)                           1 ACT (in-place)
    prod    = th * ve[s]                          1 DVE (in-place)
    e[b,n]  = sum_s prod                          1 DVE tensor_reduce
    p = exp(e); S = sum_n p; r = 1/S              1 ACT + 2 DVE
      (explicit reduce: ACT accum_out readback serializes)
    u       = p*r*x_t  (bf16)                     1 DVE
    u^T     via DMA xbar transpose                1 DMA
    z^T[g]  = W'^T u + U'^T H + b'  (4 slots)     12 PE -> PSUM [128,4,64]
      (H/bias parts depend only on last step: scheduler hoists them)
    gates in m-major: tanh(z^T), a, b2, C^T_new,
      tanh(c), H^T_new                            2 ACT + 4 DVE
    H^T lands directly in the out ring slot; no state transposes.
  out[t, m, b] = H^T (=2h^T) ring -> bf16 -> DRAM per window; host *0.5.

State H=2h, C=2c stored doubled; all x0.5 folds are in the host-prepped
weights (We_eff=We/2; W',U',b' carry the sigmoid arg/2 for i,f,o and
U'=U/2 for the H doubling; gate column order (f,i,o,g)).
X, Ue, W' ship as bf16 (halves tunnel upload); rel err ~4e-3 vs 2e-2 gate.
"""

import numpy as np

B, T, N, M = 512, 256, 128, 128
NCORES = 8
BL = B // NCORES          # 64 batch rows per core
RW = 16                   # out/x ring window (steps per DMA)

_cached = {}


def _build_nc(t_steps=T, out_t=None, skip=()):
    import concourse.bass as bass
    import concourse.tile as tile
    from concourse import bacc, mybir
    from concourse.masks import make_identity

    f32 = mybir.dt.float32
    bf16 = mybir.dt.bfloat16
    AF = mybir.ActivationFunctionType
    OP = mybir.AluOpType
    if out_t is None:
        out_t = t_steps

    nc = bacc.Bacc()

    X = nc.declare_dram_parameter("X", [BL, T, N], bf16, isOutput=False)
    We = nc.declare_dram_parameter("We", [2 * M, T], f32, isOutput=False)
    Ue = nc.declare_dram_parameter("Ue", [T, T], bf16, isOutput=False)
    brow = nc.declare_dram_parameter("brow", [T], f32, isOutput=False)
    ve = nc.declare_dram_parameter("ve", [T, 1], f32, isOutput=False)
    W_l = nc.declare_dram_parameter("W_lstm", [N, 4 * M], bf16, isOutput=False)
    U_l = nc.declare_dram_parameter("U_lstm", [M, 4 * M], f32, isOutput=False)
    b_l = nc.declare_dram_parameter("b_lstm", [4 * M], f32, isOutput=False)
    out = nc.declare_dram_parameter("out", [out_t, M, BL], bf16, isOutput=True)

    with tile.TileContext(nc) as tc:
        with tc.tile_pool(name="singles", bufs=1) as singles:
            # ---- resident weights ----
            we_sb = singles.tile([128, 2, T], f32)       # [m, {H,C}, s]
            nc.sync.dma_start(out=we_sb, in_=We.rearrange("(kt p) s -> p kt s", p=128))
            wl_sb = singles.tile([128, 4 * M], bf16)
            nc.sync.dma_start(out=wl_sb, in_=W_l[:, :])
            ul_sb = singles.tile([128, 4 * M], f32)
            nc.sync.dma_start(out=ul_sb, in_=U_l[:, :])
            blrow = singles.tile([1, 4 * M], f32)
            nc.sync.dma_start(out=blrow, in_=b_l[None, :])
            brow_sb = singles.tile([1, T], f32)
            nc.sync.dma_start(out=brow_sb, in_=brow[None, :])
            verow = singles.tile([1, T], f32)
            nc.sync.dma_start(out=verow, in_=ve.rearrange("s o -> o s"))
            ones64 = singles.tile([1, BL], f32)
            nc.vector.memset(ones64, 1.0)

            # [64]-partition broadcasts of ve and (be+bu), built via ones x row
            with tc.tile_pool(name="bc_ps", bufs=1, space="PSUM") as bc_ps:
                bc = bc_ps.tile([BL, 2, T], f32)
                nc.tensor.matmul(bc[:, 0, :], ones64, verow, start=True, stop=True)
                nc.tensor.matmul(bc[:, 1, :], ones64, brow_sb, start=True, stop=True)
                vebc = singles.tile([BL, T], f32)
                biasbc = singles.tile([BL, T], f32)
                nc.vector.tensor_copy(vebc, bc[:, 0, :])
                nc.vector.tensor_copy(biasbc, bc[:, 1, :])

            # ux resident: [b, n, s] bf16 (includes +be+bu)
            ux = singles.tile([BL, N, T], bf16)
            buf1 = singles.tile([BL, N, T], bf16)

            # ---- preamble: ux[b, n, s] = sum_t X[b, t, n] Ue[t, s] ----
            with (
                tc.tile_pool(name="pre", bufs=1) as pre,
                tc.tile_pool(name="pre_ps", bufs=4, space="PSUM") as pre_ps,
            ):
                xT = pre.tile([128, 2, BL, N], bf16)     # [t, kt, b, n]
                for kt in range(2):
                    nc.sync.dma_start(
                        out=xT[:, kt, :, :],
                        in_=X[:, kt * 128:(kt + 1) * 128, :].rearrange(
                            "b p n -> p b n"))
                ue_sb = pre.tile([128, 2, T], bf16)
                nc.sync.dma_start(
                    out=ue_sb, in_=Ue.rearrange("(kt p) s -> p kt s", p=128))
                for n0 in range(0, N, 2):                # 2 n-slices per PSUM bank
                    pq = pre_ps.tile([BL, 2, T], f32)
                    for j in range(2):
                        for kt in range(2):
                            nc.tensor.matmul(
                                pq[:, j, :],
                                xT[:, kt, :, n0 + j],
                                ue_sb[:, kt, :],
                                start=(kt == 0), stop=(kt == 1))
                    nc.vector.tensor_tensor(
                        ux[:, n0:n0 + 2, :], pq,
                        biasbc[:, None, :].broadcast_to((BL, 2, T)), OP.add)

            # ---- recurrent loop ----
            # state: H^T lives in the out ring slots (written directly in
            # m-major by the gate ops); C^T ping-pongs between two tiles
            # (even steps/window -> body starts and ends on slot 0, making
            # a For_i over windows legal)
            hT_ring = singles.tile([128, RW, BL], f32)
            nc.vector.memset(hT_ring[:, RW - 1, :], 0.0)
            CbT2 = [singles.tile([128, BL], f32, name=f"CbT{k}", tag=f"CbT{k}")
                    for k in range(2)]
            nc.vector.memset(CbT2[0], 0.0)

            with (
                tc.tile_pool(name="ring", bufs=1) as ring,
                tc.tile_pool(name="small", bufs=2) as small,
                tc.tile_pool(name="hs_ps", bufs=2, space="PSUM") as hs_ps_pool,
                tc.tile_pool(name="z_ps", bufs=2, space="PSUM") as z_ps_pool,
            ):
                def step(j, w, xr):
                    """One recurrent step; C state slot j%2 -> (j+1)%2."""
                    hTprev = hT_ring[:, (j - 1) % RW, :]
                    CbT, CbTN = CbT2[j % 2], CbT2[(j + 1) % 2]

                    # hs = [H;C] contraction: [64, 256] b-major in PSUM
                    hs_ps = hs_ps_pool.tile([BL, T], f32)
                    nc.tensor.matmul(hs_ps, hTprev, we_sb[:, 0, :],
                                     start=True, stop=False)
                    nc.tensor.matmul(hs_ps, CbT, we_sb[:, 1, :],
                                     start=False, stop=True)

                    # attention: 3 in-place wide ops + reduce
                    nc.vector.tensor_tensor(
                        buf1, ux, hs_ps[:, None, :].broadcast_to((BL, N, T)),
                        OP.add)
                    nc.scalar.activation(buf1, buf1, AF.Tanh)
                    nc.vector.tensor_tensor(
                        buf1, buf1, vebc[:, None, :].broadcast_to((BL, N, T)),
                        OP.mult)
                    e_sb = small.tile([BL, N], f32, tag="e")
                    nc.vector.tensor_reduce(e_sb, buf1, axis=mybir.AxisListType.X,
                                            op=OP.add)

                    # softmax (|e| <= ||ve||_1 ~ 13, skip max-subtract);
                    # explicit reduce for S (ACT accum_out serializes)
                    p_sb = small.tile([BL, N], f32, tag="p")
                    nc.scalar.activation(p_sb, e_sb, AF.Exp)
                    S_sb = small.tile([BL, 1], f32, tag="S")
                    nc.vector.tensor_reduce(S_sb, p_sb, axis=mybir.AxisListType.X,
                                            op=OP.add)
                    r_sb = small.tile([BL, 1], f32, tag="r")
                    nc.vector.reciprocal(r_sb, S_sb)
                    u_sb = small.tile([BL, N], bf16, tag="u")
                    nc.vector.scalar_tensor_tensor(
                        u_sb, p_sb, r_sb, xr[:, j, :], OP.mult, OP.mult)

                    # u^T via DMA xbar transpose (bf16)
                    utT = small.tile([N, BL], bf16, tag="utT")
                    nc.sync.dma_start_transpose(utT, u_sb)

                    # z^T per gate slot (f, i, o, g): [128, 4, 64] in PSUM.
                    # H/bias parts first (need only last step state: hoistable)
                    z_ps = z_ps_pool.tile([128, 4, BL], f32)
                    for g in range(4):
                        gsl = slice(g * M, (g + 1) * M)
                        nc.tensor.matmul(z_ps[:, g, :], ul_sb[:, gsl], hTprev,
                                         start=True, stop=False)
                        nc.tensor.matmul(z_ps[:, g, :], blrow[:, gsl], ones64,
                                         start=False, stop=False)
                        nc.tensor.matmul(z_ps[:, g, :], wl_sb[:, gsl], utT,
                                         start=False, stop=True)

                    # gates in m-major; sigmoid(x) = .5 + .5 tanh(x/2), /2
                    # pre-folded into W'/U'/b' for f,i,o
                    t_all = small.tile([128, 4, BL], f32, tag="tall")
                    nc.scalar.activation(t_all, z_ps, AF.Tanh)
                    a_sb = small.tile([128, BL], f32, tag="ga")
                    nc.vector.scalar_tensor_tensor(
                        a_sb, t_all[:, 0, :], 1.0, CbT, OP.add, OP.mult)
                    b2_sb = small.tile([128, BL], f32, tag="gb")
                    nc.vector.scalar_tensor_tensor(
                        b2_sb, t_all[:, 1, :], 1.0, t_all[:, 3, :],
                        OP.add, OP.mult)
                    nc.vector.scalar_tensor_tensor(
                        CbTN, a_sb, 0.5, b2_sb, OP.mult, OP.add)
                    tc_sb = small.tile([128, BL], f32, tag="tc")
                    nc.scalar.activation(tc_sb, CbTN, AF.Tanh, scale=0.5)
                    nc.vector.scalar_tensor_tensor(
                        hT_ring[:, j, :], t_all[:, 2, :], 1.0, tc_sb,
                        OP.add, OP.mult)

                def window(t0, w):
                    """w steps starting at t0 (int or For_i register)."""
                    tsl = t0 if isinstance(t0, int) else None
                    xsl = (X[:, tsl:tsl + w, :] if tsl is not None
                           else X[:, bass.ds(t0, w), :])
                    osl = (out[tsl:tsl + w] if tsl is not None
                           else out[bass.ds(t0, w)])
                    xr = ring.tile([BL, RW, N], bf16, tag="xr")
                    nc.sync.dma_start(out=xr[:, :w, :], in_=xsl)
                    for j in range(w):
                        step(j, w, xr)
                    hrbf = ring.tile([128, RW, BL], bf16, tag="hrbf")
                    nc.vector.tensor_copy(hrbf[:, :w, :], hT_ring[:, :w, :])
                    nc.sync.dma_start(
                        out=osl.rearrange("r m b -> m r b"),
                        in_=hrbf[:, :w, :])

                if t_steps % (2 * RW) == 0:
                    with tc.For_i(0, t_steps, RW) as t0:
                        window(t0, RW)
                else:
                    for t0 in range(0, t_steps, RW):
                        window(t0, min(RW, t_steps - t0))
    nc.finalize()
    return nc


def _prep_weights(inputs):
    import ml_dtypes
    bf16 = ml_dtypes.bfloat16
    We = np.asarray(inputs["We"], np.float32)
    be = np.asarray(inputs["be"], np.float32)
    bu = np.asarray(inputs["bu"], np.float32)
    Ue = np.asarray(inputs["Ue"], np.float32)
    ve = np.asarray(inputs["ve"], np.float32)
    W = np.asarray(inputs["W_lstm"], np.float32)
    U = np.asarray(inputs["U_lstm"], np.float32)
    b = np.asarray(inputs["b_lstm"], np.float32)
    perm = [1, 0, 3, 2]          # (i,f,g,o) -> (f,i,o,g)
    D = [0.5, 0.5, 0.5, 1.0]
    W_eff = np.concatenate(
        [W[:, p * M:(p + 1) * M] * d for p, d in zip(perm, D)], axis=1)
    U_eff = np.concatenate(
        [0.5 * U[:, p * M:(p + 1) * M] * d for p, d in zip(perm, D)], axis=1)
    b_eff = np.concatenate([b[p * M:(p + 1) * M] * d for p, d in zip(perm, D)])
    return {
        "We": np.ascontiguousarray(0.5 * We),
        "Ue": np.ascontiguousarray(Ue.astype(bf16)),
        "brow": np.ascontiguousarray(be + bu),
        "ve": np.ascontiguousarray(ve),
        "W_lstm": np.ascontiguousarray(W_eff.astype(bf16)),
        "U_lstm": np.ascontiguousarray(U_eff),
        "b_lstm": np.ascontiguousarray(b_eff),
    }


def kernel(_trace=False, **inputs):
    import os
    if not _trace:
        os.environ["BASS_NEVER_TRACE"] = "1"
    import ml_dtypes
    from concourse.bass_utils import run_bass_kernel_spmd

    bf16 = ml_dtypes.bfloat16
    if "nc" not in _cached:
        _cached["nc"] = _build_nc()
    nc = _cached["nc"]

    prepped = _prep_weights(inputs)
    Xb = np.asarray(inputs["X"], np.float32).astype(bf16)
    in_maps = []
    for c in range(NCORES):
        m = {"X": np.ascontiguousarray(Xb[c * BL:(c + 1) * BL])}
        m.update(prepped)
        in_maps.append(m)

    res = run_bass_kernel_spmd(nc, in_maps, core_ids=list(range(NCORES)),
                               trace=_trace)
    full = np.empty((B, T, M), np.float32)
    for c in range(NCORES):
        o = res.results[c]["out"].astype(np.float32)   # (T, M, BL), H = 2h
        full[c * BL:(c + 1) * BL] = 0.5 * o.transpose(2, 0, 1)
    return full



# revision 2
# speedup vs baseline: 1.0515x; 1.0515x over previous
"""Trainium2 Bass kernel v3 — DA-RNN input-attention encoder, packed layout.

Key change vs v2: the attention wide ops run on all 128 partitions via a
packed layout: partition p = b + 64*sh encodes batch row b (64/core) and
s-half sh (s = sh*128 + sw, T=256).  All wide ops are bf16 SBUF dense with
innermost stride 1 -> DVE 2x mode:
  buf[p, n, sw] = ux4 + hs4[p, None, :]          TT add     (2x, ~8.6us)
  buf = tanh(buf)                                ACT        (~13.8us)
  buf *= ve4[p, None, :]                         TT mult    (2x, ~8.6us)
  fold tree over sw: 64/32/16/8 + reduce         TT adds 2x (~9.3us)
  e4[p, n] -> e[b, n] via PE matmul with [I64;I64] stacked identity
The softmax + LSTM tail is the same doubled-state (H=2h, C=2c) scheme as
v2, with weights pre-packed on host (sigmoid-arg/2 folds, gate perm).
n is processed in 2 chunks so ACT tanh overlaps DVE mult/fold work.
"""

import numpy as np

B, T, N, M = 512, 256, 128, 128
NCORES = 8
BL = B // NCORES          # 64 batch rows per core
RW = 16                   # steps per window
NCH = 2                   # n-chunks per step

_cached = {}


def _build_nc(t_steps=T, out_t=None, nch=NCH):
    import concourse.bass as bass
    import concourse.tile as tile
    from concourse import bacc, mybir

    f32 = mybir.dt.float32
    bf16 = mybir.dt.bfloat16
    AF = mybir.ActivationFunctionType
    OP = mybir.AluOpType
    if out_t is None:
        out_t = t_steps

    nc = bacc.Bacc()

    X = nc.declare_dram_parameter("X", [BL, T, N], bf16, isOutput=False)
    WeP = nc.declare_dram_parameter("WeP", [128, 2, 2, 128], f32, isOutput=False)
    UeP = nc.declare_dram_parameter("UeP", [128, 2, 2, 128], bf16, isOutput=False)
    bias4 = nc.declare_dram_parameter("bias4", [128, 128], f32, isOutput=False)
    ve4 = nc.declare_dram_parameter("ve4", [128, 128], bf16, isOutput=False)
    fold2 = nc.declare_dram_parameter("fold2", [128, 64], f32, isOutput=False)
    ident64 = nc.declare_dram_parameter("ident64", [64, 64], bf16, isOutput=False)
    W_l = nc.declare_dram_parameter("W_lstm", [N, 4 * M], bf16, isOutput=False)
    U_l = nc.declare_dram_parameter("U_lstm", [M, 4 * M], f32, isOutput=False)
    b_l = nc.declare_dram_parameter("b_lstm", [4 * M], f32, isOutput=False)
    out = nc.declare_dram_parameter("out", [out_t, M, BL], bf16, isOutput=True)

    with tile.TileContext(nc) as tc:
        with tc.tile_pool(name="singles", bufs=1) as singles:
            # ---- resident constants ----
            we_sb = singles.tile([128, 2, 2, 128], f32)    # [m, hc, sh, sw]
            nc.sync.dma_start(out=we_sb, in_=WeP[:, :, :, :])
            ve_sb = singles.tile([128, 128], bf16)
            nc.sync.dma_start(out=ve_sb, in_=ve4[:, :])
            bias_sb = singles.tile([128, 128], f32)
            nc.sync.dma_start(out=bias_sb, in_=bias4[:, :])
            fold_sb = singles.tile([128, 64], f32)
            nc.sync.dma_start(out=fold_sb, in_=fold2[:, :])
            id_sb = singles.tile([64, 64], bf16)
            nc.sync.dma_start(out=id_sb, in_=ident64[:, :])
            wl_sb = singles.tile([128, 4 * M], bf16)
            nc.sync.dma_start(out=wl_sb, in_=W_l[:, :])
            ul_sb = singles.tile([128, 4 * M], f32)
            nc.sync.dma_start(out=ul_sb, in_=U_l[:, :])
            blrow = singles.tile([1, 4 * M], f32)
            nc.sync.dma_start(out=blrow, in_=b_l[None, :])
            ones64 = singles.tile([1, BL], f32)
            nc.vector.memset(ones64, 1.0)

            # ux4 resident [p=(b,sh), n, sw] bf16 (includes +be+bu)
            ux4 = singles.tile([128, N, 128], bf16)
            buf = singles.tile([128, N, 128], bf16)

            # ---- preamble: ux4[b+64*sh, n, sw] = sum_t X[b,t,n] Ue[t, sh*128+sw] ----
            with (
                tc.tile_pool(name="pre", bufs=1) as pre,
                tc.tile_pool(name="pre_ps", bufs=4, space="PSUM") as pre_ps,
            ):
                xT = pre.tile([128, 2, BL, N], bf16)     # [t, kt, b, n]
                for kt in range(2):
                    nc.sync.dma_start(
                        out=xT[:, kt, :, :],
                        in_=X[:, kt * 128:(kt + 1) * 128, :].rearrange(
                            "b p n -> p b n"))
                ue_sb = pre.tile([128, 2, 2, 128], bf16)  # [t, kt, sh, sw]
                nc.sync.dma_start(out=ue_sb, in_=UeP[:, :, :, :])
                for n0 in range(0, N, 2):
                    pq = pre_ps.tile([128, 2, 128], f32)
                    for j in range(2):
                        for sh in range(2):
                            psl = pq[sh * 64:(sh + 1) * 64, j, :]
                            for kt in range(2):
                                nc.tensor.matmul(
                                    psl,
                                    xT[:, kt, :, n0 + j],
                                    ue_sb[:, kt, sh, :],
                                    start=(kt == 0), stop=(kt == 1))
                    nc.vector.tensor_tensor(
                        ux4[:, n0:n0 + 2, :], pq,
                        bias_sb[:, None, :].broadcast_to((128, 2, 128)), OP.add)

            # ---- recurrent state ----
            hT_ring = singles.tile([128, RW, BL], f32)
            nc.vector.memset(hT_ring[:, RW - 1, :], 0.0)
            CbT2 = [singles.tile([128, BL], f32, name=f"CbT{k}", tag=f"CbT{k}")
                    for k in range(2)]
            nc.vector.memset(CbT2[0], 0.0)
            hs4 = singles.tile([128, 128], bf16)
            e4 = singles.tile([128, N], f32)

            with (
                tc.tile_pool(name="ring", bufs=2) as ring,
                tc.tile_pool(name="loopbufs", bufs=1) as loopbufs,
                tc.tile_pool(name="ps_all", bufs=1, space="PSUM") as ps_all,
            ):
                # pre-allocated double-buffered loop tiles (no per-step pool
                # traffic: SP-engine alloc/release was costing ~150us/step)
                hs_ps2 = [ps_all.tile([128, 128], f32, name=f"hsps{k}")
                          for k in range(2)]
                e_ps2 = [ps_all.tile([BL, N], f32, name=f"eps{k}")
                         for k in range(2)]
                z_ps2 = [ps_all.tile([128, 4, BL], f32, name=f"zps{k}")
                         for k in range(2)]
                tp_ps2 = [ps_all.tile([N, BL], bf16, name=f"tpps{k}")
                          for k in range(2)]
                p_sb2 = [loopbufs.tile([BL, N], f32, name=f"psb{k}")
                         for k in range(2)]
                S_sb2 = [loopbufs.tile([BL, 1], f32, name=f"Ssb{k}")
                         for k in range(2)]
                r_sb2 = [loopbufs.tile([BL, 1], f32, name=f"rsb{k}")
                         for k in range(2)]
                u_sb2 = [loopbufs.tile([BL, N], bf16, name=f"usb{k}")
                         for k in range(2)]
                utT2 = [loopbufs.tile([N, BL], bf16, name=f"utT{k}")
                        for k in range(2)]
                t_all2 = [loopbufs.tile([128, 4, BL], f32, name=f"tall{k}")
                          for k in range(2)]
                a_sb2 = [loopbufs.tile([128, BL], f32, name=f"asb{k}")
                         for k in range(2)]
                b2_sb2 = [loopbufs.tile([128, BL], f32, name=f"bsb{k}")
                          for k in range(2)]
                tc_sb2 = [loopbufs.tile([128, BL], f32, name=f"tcsb{k}")
                          for k in range(2)]

                def step(j, w, xr):
                    hTprev = hT_ring[:, (j - 1) % RW, :]
                    CbT, CbTN = CbT2[j % 2], CbT2[(j + 1) % 2]
                    k2 = j % 2

                    # hs in packed layout: [p=(b,sh), sw] PSUM
                    hs_ps = hs_ps2[k2]
                    for sh in range(2):
                        psl = hs_ps[sh * 64:(sh + 1) * 64, :]
                        nc.tensor.matmul(psl, CbT, we_sb[:, 1, sh, :],
                                         start=True, stop=False)
                        nc.tensor.matmul(psl, hTprev, we_sb[:, 0, sh, :],
                                         start=False, stop=True)

                    # hoisted LSTM parts (only need last-step state): emit now
                    # so PE runs them while DVE/ACT crunch the attention
                    z_ps = z_ps2[k2]
                    for g in range(4):
                        gsl = slice(g * M, (g + 1) * M)
                        nc.tensor.matmul(z_ps[:, g, :], ul_sb[:, gsl], hTprev,
                                         start=True, stop=False)
                        nc.tensor.matmul(z_ps[:, g, :], blrow[:, gsl], ones64,
                                         start=False, stop=False)

                    nc.vector.tensor_copy(hs4, hs_ps)

                    # wide attention ops, n-chunked
                    CW = N // nch
                    for c in range(nch):
                        nsl = slice(c * CW, (c + 1) * CW)
                        nc.vector.tensor_tensor(
                            buf[:, nsl, :], ux4[:, nsl, :],
                            hs4[:, None, :].broadcast_to((128, CW, 128)),
                            OP.add)
                        nc.scalar.activation(buf[:, nsl, :], buf[:, nsl, :],
                                             AF.Tanh)
                        nc.vector.tensor_tensor(
                            buf[:, nsl, :], buf[:, nsl, :],
                            ve_sb[:, None, :].broadcast_to((128, CW, 128)),
                            OP.mult)
                        wd = 64
                        while wd >= 8:
                            nc.vector.tensor_tensor(
                                buf[:, nsl, :wd], buf[:, nsl, :wd],
                                buf[:, nsl, wd:2 * wd], OP.add)
                            wd //= 2
                        nc.vector.tensor_reduce(
                            e4[:, nsl], buf[:, nsl, :8],
                            axis=mybir.AxisListType.X, op=OP.add)

                    # fold s-halves: e[b, n] = e4[b] + e4[b+64]
                    e_ps = e_ps2[k2]
                    nc.tensor.matmul(e_ps, fold_sb, e4, start=True, stop=True)

                    # softmax over n (|e| <= ||ve||_1 ~ 13, skip max-subtract)
                    p_sb = p_sb2[k2]
                    S_sb = S_sb2[k2]
                    nc.scalar.activation(p_sb, e_ps, AF.Exp, accum_out=S_sb)
                    r_sb = r_sb2[k2]
                    nc.vector.reciprocal(r_sb, S_sb)
                    u_sb = u_sb2[k2]
                    nc.vector.scalar_tensor_tensor(
                        u_sb, p_sb, r_sb, xr[:, j, :], OP.mult, OP.mult)

                    # u^T via PE transpose (bf16 PSUM) + ACT copy to SBUF
                    tp_ps = tp_ps2[k2]
                    nc.tensor.transpose(tp_ps, u_sb, id_sb)
                    utT = utT2[k2]
                    nc.scalar.activation(utT, tp_ps, AF.Copy)

                    # W'u parts finish the z accumulation groups
                    for g in range(4):
                        gsl = slice(g * M, (g + 1) * M)
                        nc.tensor.matmul(z_ps[:, g, :], wl_sb[:, gsl], utT,
                                         start=False, stop=True)

                    # gates in m-major; sigmoid(x) = .5 + .5 tanh(x/2), /2
                    # pre-folded into W'/U'/b' for f,i,o; order (f,i,o,g)
                    t_all = t_all2[k2]
                    nc.scalar.activation(t_all, z_ps, AF.Tanh)
                    a_sb = a_sb2[k2]
                    nc.vector.scalar_tensor_tensor(
                        a_sb, t_all[:, 0, :], 1.0, CbT, OP.add, OP.mult)
                    b2_sb = b2_sb2[k2]
                    nc.vector.scalar_tensor_tensor(
                        b2_sb, t_all[:, 1, :], 1.0, t_all[:, 3, :],
                        OP.add, OP.mult)
                    nc.vector.scalar_tensor_tensor(
                        CbTN, a_sb, 0.5, b2_sb, OP.mult, OP.add)
                    tc_sb = tc_sb2[k2]
                    nc.scalar.activation(tc_sb, CbTN, AF.Tanh, scale=0.5)
                    nc.vector.scalar_tensor_tensor(
                        hT_ring[:, j, :], t_all[:, 2, :], 1.0, tc_sb,
                        OP.add, OP.mult)

                def window(t0, w):
                    tsl = t0 if isinstance(t0, int) else None
                    xsl = (X[:, tsl:tsl + w, :] if tsl is not None
                           else X[:, bass.ds(t0, w), :])
                    osl = (out[tsl:tsl + w] if tsl is not None
                           else out[bass.ds(t0, w)])
                    xr = ring.tile([BL, RW, N], bf16, tag="xr")
                    nc.sync.dma_start(out=xr[:, :w, :], in_=xsl)
                    for j in range(w):
                        step(j, w, xr)
                    hrbf = ring.tile([128, RW, BL], bf16, tag="hrbf")
                    nc.vector.tensor_copy(hrbf[:, :w, :], hT_ring[:, :w, :])
                    nc.sync.dma_start(
                        out=osl.rearrange("r m b -> m r b"),
                        in_=hrbf[:, :w, :])

                if t_steps % (2 * RW) == 0:
                    with tc.For_i(0, t_steps, RW) as t0:
                        window(t0, RW)
                else:
                    for t0 in range(0, t_steps, RW):
                        window(t0, min(RW, t_steps - t0))
    nc.finalize()
    return nc


def _prep_weights(inputs):
    import ml_dtypes
    bf16 = ml_dtypes.bfloat16
    We = np.asarray(inputs["We"], np.float32)
    be = np.asarray(inputs["be"], np.float32)
    bu = np.asarray(inputs["bu"], np.float32)
    Ue = np.asarray(inputs["Ue"], np.float32)
    ve = np.asarray(inputs["ve"], np.float32)[:, 0]      # [T]
    W = np.asarray(inputs["W_lstm"], np.float32)
    U = np.asarray(inputs["U_lstm"], np.float32)
    b = np.asarray(inputs["b_lstm"], np.float32)

    # We packed [m, hc, sh, sw], with the H=2h fold (We/2)
    WeP = np.empty((128, 2, 2, 128), np.float32)
    Weh = 0.5 * We[:M]        # [128, 256]
    Wec = 0.5 * We[M:]
    for sh in range(2):
        WeP[:, 0, sh, :] = Weh[:, sh * 128:(sh + 1) * 128]
        WeP[:, 1, sh, :] = Wec[:, sh * 128:(sh + 1) * 128]

    # Ue packed [t, kt, sh, sw]
    UeP = np.empty((128, 2, 2, 128), np.float32)
    for kt in range(2):
        for sh in range(2):
            UeP[:, kt, sh, :] = Ue[kt * 128:(kt + 1) * 128,
                                   sh * 128:(sh + 1) * 128]

    brow = be + bu            # [T]
    bias4 = np.empty((128, 128), np.float32)
    bias4[:64, :] = brow[None, :128]
    bias4[64:, :] = brow[None, 128:]
    ve4 = np.empty((128, 128), np.float32)
    ve4[:64, :] = ve[None, :128]
    ve4[64:, :] = ve[None, 128:]
    fold2 = np.concatenate([np.eye(64, dtype=np.float32)] * 2, axis=0)

    perm = [1, 0, 3, 2]          # (i,f,g,o) -> (f,i,o,g)
    D = [0.5, 0.5, 0.5, 1.0]
    W_eff = np.concatenate(
        [W[:, p * M:(p + 1) * M] * d for p, d in zip(perm, D)], axis=1)
    U_eff = np.concatenate(
        [0.5 * U[:, p * M:(p + 1) * M] * d for p, d in zip(perm, D)], axis=1)
    b_eff = np.concatenate([b[p * M:(p + 1) * M] * d for p, d in zip(perm, D)])
    return {
        "WeP": np.ascontiguousarray(WeP),
        "UeP": np.ascontiguousarray(UeP.astype(bf16)),
        "bias4": np.ascontiguousarray(bias4),
        "ve4": np.ascontiguousarray(ve4.astype(bf16)),
        "fold2": np.ascontiguousarray(fold2),
        "ident64": np.ascontiguousarray(np.eye(64, dtype=np.float32).astype(bf16)),
        "W_lstm": np.ascontiguousarray(W_eff.astype(bf16)),
        "U_lstm": np.ascontiguousarray(U_eff),
        "b_lstm": np.ascontiguousarray(b_eff),
    }


def kernel(_trace=False, _t_steps=T, **inputs):
    import os
    if not _trace:
        os.environ["BASS_NEVER_TRACE"] = "1"
    import ml_dtypes
    from concourse.bass_utils import run_bass_kernel_spmd

    bf16 = ml_dtypes.bfloat16
    key = ("nc", _t_steps)
    if key not in _cached:
        _cached[key] = _build_nc(t_steps=_t_steps)
    nc = _cached[key]

    prepped = _prep_weights(inputs)
    Xb = np.asarray(inputs["X"], np.float32).astype(bf16)
    in_maps = []
    for c in range(NCORES):
        m = {"X": np.ascontiguousarray(Xb[c * BL:(c + 1) * BL])}
        m.update(prepped)
        in_maps.append(m)

    res = run_bass_kernel_spmd(nc, in_maps, core_ids=list(range(NCORES)),
                               trace=_trace)
    full = np.empty((B, _t_steps, M), np.float32)
    for c in range(NCORES):
        o = res.results[c]["out"].astype(np.float32)   # (t, M, BL), H = 2h
        full[c * BL:(c + 1) * BL] = 0.5 * o.transpose(2, 0, 1)
    return full


# revision 6
# speedup vs baseline: 3.3035x; 3.1416x over previous
"""Trainium2 Bass kernel v3 — DA-RNN input-attention encoder, packed layout.

Measured cost model on these axon-tunneled cores (microbenched + ablated):
- cross-engine handoff ~1.1us; small engine op ~0.5-1.1us; in-loop
  per-instruction overhead ~1-3us => MINIMIZE INSTRUCTION COUNT (NCH=1:
  unchunked wide ops beat 2/4-way n-chunking by 1.8x).
- a DMA op on the serial chain costs ~86us latency (ring round-trip) even
  though its pipelined throughput is ~2us => transpose u on the PE
  (nc.tensor.transpose + identity), never dma_start_transpose per step.
- tc.tile_pool .tile() calls inside the loop emit SP-engine alloc/release
  that cost ~100us+/step => pre-allocate ALL loop tiles once, ping-pong
  manually by j%2.
- 8-core SPMD execution is ~serialized by the tunnel: metric ~ 8x per-core
  exec. Full-vs-1step wall diffs have ±100ms program-instance noise; use
  interleaved multi-program slope benches (32 vs 256 steps) to compare.
- DVE wide ops [128p, 16384f] bf16 SBUF: tensor_tensor ~9us (2x mode,
  broadcast operand OK), tensor_reduce ~18.5us (1x) => pairwise fold tree
  (~9.7us) instead of one reduce. ACT tanh/exp ~15.2us (no 2x for bf16).
  GPSIMD is 4x slower than DVE - useless.

Layout: partition p = b + 64*sh packs batch row b (64/core) and s-half sh
(s = sh*128 + sw, T=256) => all attention wide ops run on 128 partitions:
  buf[p, n, sw] = ux4 + hs4[p, None, :]   TT add (bf16 2x)
  buf = tanh(buf); buf *= ve4[p, None, :] ACT; TT mult
  fold tree 64/32/16 + reduce-8 over sw -> e4[p, n]
  e4 -> e[b, n] via PE matmul with [I64;I64]; hs4 comes from 4 PE matmuls
  writing PSUM at partition bases 0/64 (We pre-packed host-side).
Softmax: exp with fused accum_out=S, recip, STT u = p*r*x_t; u^T via PE
transpose; LSTM: 12 PE matmuls (U'h+bias hoisted before the wides, W'u
after), one tanh over [128, 4, 64] gates, 3 STT cell ops + tanh + STT.
Doubled state H=2h, C=2c with all 0.5 folds in host-prepped weights
(sigmoid(x) = .5+.5tanh(x/2); gate order (f,i,o,g)); X/Ue/W' ship bf16.
rel err ~5.4e-3 vs 2e-2 gate; ~28us/step/core (was ~830us in v2).
"""

import numpy as np

B, T, N, M = 512, 256, 128, 128
NCORES = 8
BL = B // NCORES          # 64 batch rows per core
RW = 16                   # steps per window
NCH = 1

_cached = {}


def _build_nc(t_steps=T, out_t=None, nch=NCH):
    import concourse.bass as bass
    import concourse.tile as tile
    from concourse import bacc, mybir

    f32 = mybir.dt.float32
    bf16 = mybir.dt.bfloat16
    AF = mybir.ActivationFunctionType
    OP = mybir.AluOpType
    if out_t is None:
        out_t = t_steps

    nc = bacc.Bacc()

    X = nc.declare_dram_parameter("X", [BL, T, N], bf16, isOutput=False)
    WeP = nc.declare_dram_parameter("WeP", [128, 2, 2, 128], f32, isOutput=False)
    UeP = nc.declare_dram_parameter("UeP", [128, 2, 2, 128], bf16, isOutput=False)
    bias4 = nc.declare_dram_parameter("bias4", [128, 128], f32, isOutput=False)
    ve4 = nc.declare_dram_parameter("ve4", [128, 128], bf16, isOutput=False)
    fold2 = nc.declare_dram_parameter("fold2", [128, 64], f32, isOutput=False)
    ident64 = nc.declare_dram_parameter("ident64", [64, 64], bf16, isOutput=False)
    W_l = nc.declare_dram_parameter("W_lstm", [N, 4 * M], bf16, isOutput=False)
    U_l = nc.declare_dram_parameter("U_lstm", [M, 4 * M], f32, isOutput=False)
    b_l = nc.declare_dram_parameter("b_lstm", [4 * M], f32, isOutput=False)
    out = nc.declare_dram_parameter("out", [out_t, M, BL], bf16, isOutput=True)

    with tile.TileContext(nc) as tc:
        with tc.tile_pool(name="singles", bufs=1) as singles:
            # ---- resident constants ----
            we_sb = singles.tile([128, 2, 2, 128], f32)    # [m, hc, sh, sw]
            nc.sync.dma_start(out=we_sb, in_=WeP[:, :, :, :])
            ve_sb = singles.tile([128, 128], bf16)
            nc.sync.dma_start(out=ve_sb, in_=ve4[:, :])
            bias_sb = singles.tile([128, 128], f32)
            nc.sync.dma_start(out=bias_sb, in_=bias4[:, :])
            fold_sb = singles.tile([128, 64], f32)
            nc.sync.dma_start(out=fold_sb, in_=fold2[:, :])
            id_sb = singles.tile([64, 64], bf16)
            nc.sync.dma_start(out=id_sb, in_=ident64[:, :])
            wl_sb = singles.tile([128, 4 * M], bf16)
            nc.sync.dma_start(out=wl_sb, in_=W_l[:, :])
            ul_sb = singles.tile([128, 4 * M], f32)
            nc.sync.dma_start(out=ul_sb, in_=U_l[:, :])
            blrow = singles.tile([1, 4 * M], f32)
            nc.sync.dma_start(out=blrow, in_=b_l[None, :])
            ones64 = singles.tile([1, BL], f32)
            nc.vector.memset(ones64, 1.0)

            # ux4 resident [p=(b,sh), n, sw] bf16 (includes +be+bu)
            ux4 = singles.tile([128, N, 128], bf16)
            buf = singles.tile([128, N, 128], bf16)

            # ---- preamble: ux4[b+64*sh, n, sw] = sum_t X[b,t,n] Ue[t, sh*128+sw] ----
            with (
                tc.tile_pool(name="pre", bufs=1) as pre,
                tc.tile_pool(name="pre_ps", bufs=4, space="PSUM") as pre_ps,
            ):
                xT = pre.tile([128, 2, BL, N], bf16)     # [t, kt, b, n]
                for kt in range(2):
                    nc.sync.dma_start(
                        out=xT[:, kt, :, :],
                        in_=X[:, kt * 128:(kt + 1) * 128, :].rearrange(
                            "b p n -> p b n"))
                ue_sb = pre.tile([128, 2, 2, 128], bf16)  # [t, kt, sh, sw]
                nc.sync.dma_start(out=ue_sb, in_=UeP[:, :, :, :])
                for n0 in range(0, N, 2):
                    pq = pre_ps.tile([128, 2, 128], f32)
                    for j in range(2):
                        for sh in range(2):
                            psl = pq[sh * 64:(sh + 1) * 64, j, :]
                            for kt in range(2):
                                nc.tensor.matmul(
                                    psl,
                                    xT[:, kt, :, n0 + j],
                                    ue_sb[:, kt, sh, :],
                                    start=(kt == 0), stop=(kt == 1))
                    nc.vector.tensor_tensor(
                        ux4[:, n0:n0 + 2, :], pq,
                        bias_sb[:, None, :].broadcast_to((128, 2, 128)), OP.add)

            # ---- recurrent state ----
            hT_ring = singles.tile([128, RW, BL], f32)
            nc.vector.memset(hT_ring[:, RW - 1, :], 0.0)
            CbT2 = [singles.tile([128, BL], f32, name=f"CbT{k}", tag=f"CbT{k}")
                    for k in range(2)]
            nc.vector.memset(CbT2[0], 0.0)
            hs4 = singles.tile([128, 128], bf16)
            e4 = singles.tile([128, N], f32)

            with (
                tc.tile_pool(name="ring", bufs=2) as ring,
                tc.tile_pool(name="loopbufs", bufs=1) as loopbufs,
                tc.tile_pool(name="ps_all", bufs=1, space="PSUM") as ps_all,
            ):
                # pre-allocated double-buffered loop tiles (no per-step pool
                # traffic: SP-engine alloc/release was costing ~150us/step)
                hs_ps2 = [ps_all.tile([128, 128], f32, name=f"hsps{k}")
                          for k in range(2)]
                e_ps2 = [ps_all.tile([BL, N], f32, name=f"eps{k}")
                         for k in range(2)]
                z_ps2 = [ps_all.tile([128, 4, BL], f32, name=f"zps{k}")
                         for k in range(2)]
                tp_ps2 = [ps_all.tile([N, BL], bf16, name=f"tpps{k}")
                          for k in range(2)]
                p_sb2 = [loopbufs.tile([BL, N], f32, name=f"psb{k}")
                         for k in range(2)]
                S_sb2 = [loopbufs.tile([BL, 1], f32, name=f"Ssb{k}")
                         for k in range(2)]
                r_sb2 = [loopbufs.tile([BL, 1], f32, name=f"rsb{k}")
                         for k in range(2)]
                u_sb2 = [loopbufs.tile([BL, N], bf16, name=f"usb{k}")
                         for k in range(2)]
                utT2 = [loopbufs.tile([N, BL], bf16, name=f"utT{k}")
                        for k in range(2)]
                t_all2 = [loopbufs.tile([128, 4, BL], f32, name=f"tall{k}")
                          for k in range(2)]
                a_sb2 = [loopbufs.tile([128, BL], f32, name=f"asb{k}")
                         for k in range(2)]
                b2_sb2 = [loopbufs.tile([128, BL], f32, name=f"bsb{k}")
                          for k in range(2)]
                tc_sb2 = [loopbufs.tile([128, BL], f32, name=f"tcsb{k}")
                          for k in range(2)]

                def step(j, w, xr):
                    hTprev = hT_ring[:, (j - 1) % RW, :]
                    CbT, CbTN = CbT2[j % 2], CbT2[(j + 1) % 2]
                    k2 = j % 2

                    # hs in packed layout: [p=(b,sh), sw] PSUM
                    hs_ps = hs_ps2[k2]
                    for sh in range(2):
                        psl = hs_ps[sh * 64:(sh + 1) * 64, :]
                        nc.tensor.matmul(psl, CbT, we_sb[:, 1, sh, :],
                                         start=True, stop=False)
                        nc.tensor.matmul(psl, hTprev, we_sb[:, 0, sh, :],
                                         start=False, stop=True)

                    # hoisted LSTM parts (only need last-step state): emit now
                    # so PE runs them while DVE/ACT crunch the attention
                    z_ps = z_ps2[k2]
                    for g in range(4):
                        gsl = slice(g * M, (g + 1) * M)
                        nc.tensor.matmul(z_ps[:, g, :], ul_sb[:, gsl], hTprev,
                                         start=True, stop=False)
                        nc.tensor.matmul(z_ps[:, g, :], blrow[:, gsl], ones64,
                                         start=False, stop=False)

                    nc.vector.tensor_copy(hs4, hs_ps)

                    # wide attention ops, n-chunked
                    CW = N // nch
                    for c in range(nch):
                        nsl = slice(c * CW, (c + 1) * CW)
                        nc.vector.tensor_tensor(
                            buf[:, nsl, :], ux4[:, nsl, :],
                            hs4[:, None, :].broadcast_to((128, CW, 128)),
                            OP.add)
                        nc.scalar.activation(buf[:, nsl, :], buf[:, nsl, :],
                                             AF.Tanh)
                        nc.vector.tensor_tensor(
                            buf[:, nsl, :], buf[:, nsl, :],
                            ve_sb[:, None, :].broadcast_to((128, CW, 128)),
                            OP.mult)
                        wd = 64
                        while wd >= 8:
                            nc.vector.tensor_tensor(
                                buf[:, nsl, :wd], buf[:, nsl, :wd],
                                buf[:, nsl, wd:2 * wd], OP.add)
                            wd //= 2
                        nc.vector.tensor_reduce(
                            e4[:, nsl], buf[:, nsl, :8],
                            axis=mybir.AxisListType.X, op=OP.add)

                    # fold s-halves: e[b, n] = e4[b] + e4[b+64]
                    e_ps = e_ps2[k2]
                    nc.tensor.matmul(e_ps, fold_sb, e4, start=True, stop=True)

                    # softmax over n (|e| <= ||ve||_1 ~ 13, skip max-subtract)
                    p_sb = p_sb2[k2]
                    S_sb = S_sb2[k2]
                    nc.scalar.activation(p_sb, e_ps, AF.Exp, accum_out=S_sb)
                    r_sb = r_sb2[k2]
                    nc.vector.reciprocal(r_sb, S_sb)
                    u_sb = u_sb2[k2]
                    nc.vector.scalar_tensor_tensor(
                        u_sb, p_sb, r_sb, xr[:, j, :], OP.mult, OP.mult)

                    # u^T via PE transpose (bf16 PSUM) + ACT copy to SBUF
                    tp_ps = tp_ps2[k2]
                    nc.tensor.transpose(tp_ps, u_sb, id_sb)
                    utT = utT2[k2]
                    nc.scalar.activation(utT, tp_ps, AF.Copy)

                    # W'u parts finish the z accumulation groups
                    for g in range(4):
                        gsl = slice(g * M, (g + 1) * M)
                        nc.tensor.matmul(z_ps[:, g, :], wl_sb[:, gsl], utT,
                                         start=False, stop=True)

                    # gates in m-major; sigmoid(x) = .5 + .5 tanh(x/2), /2
                    # pre-folded into W'/U'/b' for f,i,o; order (f,i,o,g)
                    t_all = t_all2[k2]
                    nc.scalar.activation(t_all, z_ps, AF.Tanh)
                    a_sb = a_sb2[k2]
                    nc.vector.scalar_tensor_tensor(
                        a_sb, t_all[:, 0, :], 1.0, CbT, OP.add, OP.mult)
                    b2_sb = b2_sb2[k2]
                    nc.vector.scalar_tensor_tensor(
                        b2_sb, t_all[:, 1, :], 1.0, t_all[:, 3, :],
                        OP.add, OP.mult)
                    nc.vector.scalar_tensor_tensor(
                        CbTN, a_sb, 0.5, b2_sb, OP.mult, OP.add)
                    tc_sb = tc_sb2[k2]
                    nc.scalar.activation(tc_sb, CbTN, AF.Tanh, scale=0.5)
                    nc.vector.scalar_tensor_tensor(
                        hT_ring[:, j, :], t_all[:, 2, :], 1.0, tc_sb,
                        OP.add, OP.mult)

                def window(t0, w):
                    tsl = t0 if isinstance(t0, int) else None
                    xsl = (X[:, tsl:tsl + w, :] if tsl is not None
                           else X[:, bass.ds(t0, w), :])
                    osl = (out[tsl:tsl + w] if tsl is not None
                           else out[bass.ds(t0, w)])
                    xr = ring.tile([BL, RW, N], bf16, tag="xr")
                    nc.sync.dma_start(out=xr[:, :w, :], in_=xsl)
                    for j in range(w):
                        step(j, w, xr)
                    hrbf = ring.tile([128, RW, BL], bf16, tag="hrbf")
                    nc.vector.tensor_copy(hrbf[:, :w, :], hT_ring[:, :w, :])
                    nc.sync.dma_start(
                        out=osl.rearrange("r m b -> m r b"),
                        in_=hrbf[:, :w, :])

                if t_steps % (2 * RW) == 0:
                    with tc.For_i(0, t_steps, RW) as t0:
                        window(t0, RW)
                else:
                    for t0 in range(0, t_steps, RW):
                        window(t0, min(RW, t_steps - t0))
    nc.finalize()
    return nc


def _prep_weights(inputs):
    import ml_dtypes
    bf16 = ml_dtypes.bfloat16
    We = np.asarray(inputs["We"], np.float32)
    be = np.asarray(inputs["be"], np.float32)
    bu = np.asarray(inputs["bu"], np.float32)
    Ue = np.asarray(inputs["Ue"], np.float32)
    ve = np.asarray(inputs["ve"], np.float32)[:, 0]      # [T]
    W = np.asarray(inputs["W_lstm"], np.float32)
    U = np.asarray(inputs["U_lstm"], np.float32)
    b = np.asarray(inputs["b_lstm"], np.float32)

    # We packed [m, hc, sh, sw], with the H=2h fold (We/2)
    WeP = np.empty((128, 2, 2, 128), np.float32)
    Weh = 0.5 * We[:M]        # [128, 256]
    Wec = 0.5 * We[M:]
    for sh in range(2):
        WeP[:, 0, sh, :] = Weh[:, sh * 128:(sh + 1) * 128]
        WeP[:, 1, sh, :] = Wec[:, sh * 128:(sh + 1) * 128]

    # Ue packed [t, kt, sh, sw]
    UeP = np.empty((128, 2, 2, 128), np.float32)
    for kt in range(2):
        for sh in range(2):
            UeP[:, kt, sh, :] = Ue[kt * 128:(kt + 1) * 128,
                                   sh * 128:(sh + 1) * 128]

    brow = be + bu            # [T]
    bias4 = np.empty((128, 128), np.float32)
    bias4[:64, :] = brow[None, :128]
    bias4[64:, :] = brow[None, 128:]
    ve4 = np.empty((128, 128), np.float32)
    ve4[:64, :] = ve[None, :128]
    ve4[64:, :] = ve[None, 128:]
    fold2 = np.concatenate([np.eye(64, dtype=np.float32)] * 2, axis=0)

    perm = [1, 0, 3, 2]          # (i,f,g,o) -> (f,i,o,g)
    D = [0.5, 0.5, 0.5, 1.0]
    W_eff = np.concatenate(
        [W[:, p * M:(p + 1) * M] * d for p, d in zip(perm, D)], axis=1)
    U_eff = np.concatenate(
        [0.5 * U[:, p * M:(p + 1) * M] * d for p, d in zip(perm, D)], axis=1)
    b_eff = np.concatenate([b[p * M:(p + 1) * M] * d for p, d in zip(perm, D)])
    return {
        "WeP": np.ascontiguousarray(WeP),
        "UeP": np.ascontiguousarray(UeP.astype(bf16)),
        "bias4": np.ascontiguousarray(bias4),
        "ve4": np.ascontiguousarray(ve4.astype(bf16)),
        "fold2": np.ascontiguousarray(fold2),
        "ident64": np.ascontiguousarray(np.eye(64, dtype=np.float32).astype(bf16)),
        "W_lstm": np.ascontiguousarray(W_eff.astype(bf16)),
        "U_lstm": np.ascontiguousarray(U_eff),
        "b_lstm": np.ascontiguousarray(b_eff),
    }


def kernel(_trace=False, _t_steps=T, **inputs):
    import os
    if not _trace:
        os.environ["BASS_NEVER_TRACE"] = "1"
    import ml_dtypes
    from concourse.bass_utils import run_bass_kernel_spmd

    bf16 = ml_dtypes.bfloat16
    key = ("nc", _t_steps)
    if key not in _cached:
        _cached[key] = _build_nc(t_steps=_t_steps)
    nc = _cached[key]

    prepped = _prep_weights(inputs)
    Xb = np.asarray(inputs["X"], np.float32).astype(bf16)
    in_maps = []
    for c in range(NCORES):
        m = {"X": np.ascontiguousarray(Xb[c * BL:(c + 1) * BL])}
        m.update(prepped)
        in_maps.append(m)

    res = run_bass_kernel_spmd(nc, in_maps, core_ids=list(range(NCORES)),
                               trace=_trace)
    full = np.empty((B, _t_steps, M), np.float32)
    for c in range(NCORES):
        o = res.results[c]["out"].astype(np.float32)   # (t, M, BL), H = 2h
        full[c * BL:(c + 1) * BL] = 0.5 * o.transpose(2, 0, 1)
    return full
